# revision 73
# baseline (speedup 1.0000x reference)
"""Multi-head attention kernel for Trainium2, sharded over 8 NeuronCores.

Problem: x[2,2048,1024] -> MHA(16 heads, dh=64) -> out[2,2048,512].

Sharding: core c handles batch b=c//4 and head-group g=c%4 (4 heads each).
Each core computes QKV for its heads, attention, and a partial output
projection through its 256-row slice of Wo. Host sums the 4 head-group
partials per batch and adds bo + bv@Wo (the V bias commutes out of the
softmax-weighted sum, so it is folded into a host-side constant).

Per-core kernel design (all matmuls bf16 operands, fp32 PSUM accumulate):
  - x^T [din, s] arrives pre-transposed from the host (contraction for
    QKV is din), streamed by q-chunk so projections start on first bytes.
  - Q^T, K^T packed in one [128, q/k, pair, s] tile: head h at partition
    base 64*(h%2); scores^T tiles [k,q] come from lhsT=K^T slice,
    rhs=Q^T slice at the same base (distinct PE row-groups per head).
  - V stored natural [s, (head, dh)] (no ones column needed).
  - softmax: exp on ScalarE with scale=1/8 folded in, bf16 output; no max
    subtraction (scores are bounded ~|2| for these inputs).
  - attention in NATURAL layout: lhsT = exp(S^T) [k, q-tile], rhs = V
    [k, 64] -> psum [q-tile, 64] in 64 PE cycles/instr (the PE cost model
    charges output free size, so this halves attention PE time vs the
    attn^T orientation). Denominators ride 1-cycle ones-column matmuls
    into a [q, (j,qt)] psum accumulator.
  - normalization: DVE reciprocal of the denominators (q on partitions ->
    native per-partition broadcast), per-q-tile multiply into a bf16
    staging tile [q, j0|j1], then a PE transpose (128 cycles) lands
    attn^T [dq-pair, q] for the output projection.
  - out partial [s, 512] = attnT.T @ Wo_slice via lhsT=attnT tiles.
  - Schedule: a unified software pipeline over 64 (head-pair, q-chunk, qq)
    units. ScalarE's exp stream (the ~134us co-bottleneck with PE) is
    emitted at one unit per driver step and runs nearly gapless; the
    attention matmuls lag it by 12 units initially (shedding deferrable PE
    work out of the DMA/projection-gated lead-in, exp tiles buffering in
    SBUF) and catch up to a lag of 3 through double-attention steps in the
    ACT-bound middle. V projections, K/Q m1 and deferred Q m0 projections
    are split into <=1us pieces placed at just-in-time steps after each
    s_exp so scores never queue behind them; out-projections trail each
    at_sb q-chunk completion, with the final four riding the last finish
    (whose normalization is split DVE/ScalarE since the exp stream is done
    by then). Ten throwaway matmuls at t=0 burn the PE's 3us p-state ramp
    during the initial DMA wait.
"""

import sys

sys.path.insert(0, "/opt/trn_rl_repo")

import numpy as np
from contextlib import ExitStack

# Problem shapes (hardcoded per the harness contract).
B = 2
S = 2048
DIN = 1024
H = 16
DH = 64
DMODEL = H * DH  # 1024
DOUT = 512
NCORES = 8

# Per-core shard shapes.
HPC = 4  # heads per core
DQ = HPC * DH  # 256: per-core QKV width
KT = DIN // 128  # 8  k-tiles over d_in
MT = DQ // 128  # 2  m-tiles over per-core dq
ST = S // 128  # 16 s-tiles
QC = S // 512  # 4  q-chunks of 512
KC = S // 128  # 16 k-tiles over sequence


def build_program(repeat=1):
    from concourse import bacc, tile
    import concourse.bass as bass
    import concourse.mybir as mybir

    f32 = mybir.dt.float32
    bf16 = mybir.dt.bfloat16
    Exp = mybir.ActivationFunctionType.Exp

    nc = bacc.Bacc("TRN2", target_bir_lowering=False, debug=False)

    x_d = nc.dram_tensor("x", [QC, 128, KT, 512], bf16, kind="ExternalInput")
    # Wq/Wk are m-major so each 128-column half is one contiguous-per-
    # partition DMA (2KB runs; sub-512B runs pay a 2x DMA latency penalty).
    wq_d = nc.dram_tensor("wq", [MT, 128, KT, 128], bf16, kind="ExternalInput")
    wk_d = nc.dram_tensor("wk", [MT, 128, KT, 128], bf16, kind="ExternalInput")
    wv_d = nc.dram_tensor("wv", [128, KT, DQ], bf16, kind="ExternalInput")
    bq_d = nc.dram_tensor("bq", [DH, HPC], f32, kind="ExternalInput")
    bk_d = nc.dram_tensor("bk", [DH, HPC], f32, kind="ExternalInput")
    wo_d = nc.dram_tensor("wo", [128, MT, DOUT], bf16, kind="ExternalInput")
    id_d = nc.dram_tensor("ident", [128, 128], bf16, kind="ExternalInput")
    out_d = nc.dram_tensor("out", [S, DOUT], f32, kind="ExternalOutput")

    with tile.TileContext(nc) as tc, ExitStack() as octx:
        # Const DMAs are issued inside the driver's first steps (after the
        # chunk-0 critical loads) - only the tiles are allocated here.
        consts = octx.enter_context(tc.tile_pool(name="consts", bufs=1))
        ident = consts.tile([128, 128], bf16)
        onescol = consts.tile([128, 1], bf16)
        nc.vector.memset(onescol[:], 1.0)
        bq_sb = consts.tile([DH, HPC], f32)
        bk_sb = consts.tile([DH, HPC], f32)
        wo_sb = consts.tile([128, MT, DOUT], bf16)

        # Persistent intermediates. Q^T and K^T share one full-partition
        # tile: head h lives at partition base 64*(h%2), pair index h//2.
        # An S^T matmul then has lhsT (K^T) and rhs (Q^T) at the SAME base
        # partition, which bass requires (and maps to PE row-groups).
        keep = octx.enter_context(tc.tile_pool(name="keep", bufs=1))
        qk_sb = keep.tile([128, 2, MT, S], bf16)  # [part, q/k, pair, s]
        v_sb = keep.tile([128, ST, DQ], bf16)  # V natural [s, (head, dh)]
        at_sb = keep.tile([128, MT, S], bf16)  # attn^T (dq on partitions)

        for _rep in range(repeat):
            with ExitStack() as p12:
                xt_pool = p12.enter_context(tc.tile_pool(name="xt", bufs=1))
                xt_sb = xt_pool.tile([128, KT, S], bf16)  # x^T

                wts = p12.enter_context(tc.tile_pool(name="wts", bufs=1))
                wq_sb = wts.tile([128, MT, KT, 128], bf16)
                wk_sb = wts.tile([128, MT, KT, 128], bf16)
                wv_sb = wts.tile([128, KT, DQ], bf16)

                proj_ps = p12.enter_context(
                    tc.tile_pool(name="proj_ps", bufs=2, space="PSUM")
                )

                exps = p12.enter_context(tc.tile_pool(name="exps", bufs=16))
                small = p12.enter_context(tc.tile_pool(name="small", bufs=4))
                nat = p12.enter_context(tc.tile_pool(name="nat", bufs=4))
                s_ps = p12.enter_context(
                    tc.tile_pool(name="s_ps", bufs=2, space="PSUM")
                )
                a_ps = p12.enter_context(
                    tc.tile_pool(name="a_ps", bufs=1, space="PSUM")
                )
                dn_ps = p12.enter_context(
                    tc.tile_pool(name="dn_ps", bufs=1, space="PSUM")
                )
                o_sb = p12.enter_context(tc.tile_pool(name="o_sb", bufs=3))

                def qk_proj(w_sb, b_sb, qki, m, qc):
                    """One q-chunk of the Q^T (qki=0) / K^T (qki=1) m-tile."""
                    ps = proj_ps.tile([128, 512], f32, tag="proj")
                    for k in range(KT):
                        nc.tensor.matmul(
                            ps[:],
                            w_sb[:, m, k, :],
                            xt_sb[:, k, qc * 512 : (qc + 1) * 512],
                            start=(k == 0),
                            stop=(k == KT - 1),
                        )
                    for j in range(2):
                        h = 2 * m + j
                        nc.vector.tensor_scalar_add(
                            qk_sb[
                                j * 64 : j * 64 + 64,
                                qki,
                                m,
                                qc * 512 : (qc + 1) * 512,
                            ],
                            ps[j * 64 : j * 64 + 64, :],
                            b_sb[:, h : h + 1],
                        )

                def v_proj_st(st):
                    """V rows for s-tile st (no bias: bv folds into host add)."""
                    ps = proj_ps.tile([128, 512], f32, tag="proj")
                    for k in range(KT):
                        nc.tensor.matmul(
                            ps[:, :DQ],
                            xt_sb[:, k, st * 128 : (st + 1) * 128],
                            wv_sb[:, k, :],
                            start=(k == 0),
                            stop=(k == KT - 1),
                        )
                    nc.vector.tensor_copy(v_sb[:, st, :], ps[:, :DQ])

                class AttnPair:
                    """Both heads of pair p (bases 0 and 64) for q-chunk qc.

                    Emitted in eighths of 2 sequence k-tiles: both heads' S
                    matmuls (adjacent, distinct PE row-groups via their base
                    partitions), a paired 2-bank exp per head on ScalarE,
                    then the eighth's natural-layout attention matmuls with
                    1-cycle denominator matmuls riding along."""

                    def __init__(self, p, qc):
                        self.p, self.qc = p, qc
                        self.ets = {}
                        self.qsl = slice(qc * 512, (qc + 1) * 512)
                        self.aps = a_ps.tile([128, 2, 4, DH], f32, tag="a")
                        self.dns = dn_ps.tile([128, 2, 4], f32, tag="dn")

                    def s_exp(self, qq):
                        p = self.p
                        et = exps.tile([128, 2, 2, 512], bf16, tag="exps")
                        self.ets[qq] = et
                        for j in range(2):
                            base = 64 * j
                            sp = s_ps.tile([128, 2, 512], f32, tag="s")
                            for i in range(2):
                                kt = 2 * qq + i
                                nc.tensor.matmul(
                                    sp[:, i, :],
                                    qk_sb[
                                        base : base + 64,
                                        1,
                                        p,
                                        kt * 128 : (kt + 1) * 128,
                                    ],
                                    qk_sb[base : base + 64, 0, p, self.qsl],
                                    start=True,
                                    stop=True,
                                )
                            nc.scalar.activation(
                                et[:, j, :, :],
                                sp[:],
                                Exp,
                                scale=1.0 / np.sqrt(DH),
                            )

                    def attn(self, qq):
                        # The 8 (j, qt) accumulation groups share one psum
                        # bank (and the 8 denominator groups another). PSUM
                        # start=True lazily zero-marks the WHOLE 2KB bank, so
                        # only the first group may carry start (its mark
                        # covers everyone's first write) and only the last
                        # group's final matmul carries stop.
                        et = self.ets.pop(qq)
                        for i in range(2):
                            kt = 2 * qq + i
                            first, last = (kt == 0), (kt == KC - 1)
                            for j in range(2):
                                h = 2 * self.p + j
                                for qt in range(4):
                                    g = 4 * j + qt
                                    lhsT = et[
                                        :, j, i, qt * 128 : (qt + 1) * 128
                                    ]
                                    nc.tensor.matmul(
                                        self.aps[:, j, qt, :],
                                        lhsT,
                                        v_sb[:, kt, h * DH : (h + 1) * DH],
                                        start=(first and g == 0),
                                        stop=(last and g == 7),
                                        skip_group_check=True,
                                    )
                                    nc.tensor.matmul(
                                        self.dns[:, j, qt : qt + 1],
                                        lhsT,
                                        onescol[:],
                                        start=(first and g == 0),
                                        stop=(last and g == 7),
                                        skip_group_check=True,
                                    )

                    def finish(self, followers=None, act_assist=False):
                        # (GPSIMD cannot access PSUM on TRN2, so the
                        # normalization stays on DVE; for the LAST block the
                        # exp stream is over, so ScalarE takes half the
                        # multiplies to shorten the tail's critical chain.)
                        rec = small.tile([128, 2, 4], f32, tag="rec")
                        nc.vector.reciprocal(rec[:], self.dns[:])
                        for qt in range(4):
                            nat_t = nat.tile([128, 2, DH], bf16, tag="nat")
                            for j in range(2):
                                if act_assist and j == 1:
                                    nc.scalar.mul(
                                        nat_t[:, j, :],
                                        self.aps[:, j, qt, :],
                                        rec[:, j, qt : qt + 1],
                                    )
                                else:
                                    nc.vector.tensor_scalar_mul(
                                        nat_t[:, j, :],
                                        self.aps[:, j, qt, :],
                                        rec[:, j, qt : qt + 1],
                                    )
                            tp = proj_ps.tile([128, 128], bf16, tag="proj")
                            nc.tensor.transpose(
                                tp[:],
                                nat_t[:].rearrange("p a b -> p (a b)"),
                                ident[:],
                            )
                            q0 = self.qc * 512 + qt * 128
                            nc.vector.tensor_copy(
                                at_sb[:, self.p, q0 : q0 + 128], tp[:]
                            )
                            if followers:
                                followers[qt]()

                def out_proj_m(m, act_copy=False):
                    """Output partial for s-tile m."""
                    ps = proj_ps.tile([128, DOUT], f32, tag="proj")
                    for k2 in range(MT):
                        nc.tensor.matmul(
                            ps[:],
                            at_sb[:, k2, m * 128 : (m + 1) * 128],
                            wo_sb[:, k2, :],
                            start=(k2 == 0),
                            stop=(k2 == MT - 1),
                        )
                    ot = o_sb.tile([128, DOUT], f32, tag="ot")
                    if act_copy:
                        nc.scalar.copy(ot[:], ps[:])
                    else:
                        nc.vector.tensor_copy(ot[:], ps[:])
                    nc.sync.dma_start(out_d[m * 128 : (m + 1) * 128, :], ot[:])

                def KQ_halves(w_sb, b_sb, qki, m, qc):
                    """qk_proj split into two ~0.85us emission pieces that
                    share one psum tile. The scheduler must not let any
                    other proj-tag allocation land between h1 and h2."""
                    state = {}
                    qsl = slice(qc * 512, (qc + 1) * 512)

                    def h1():
                        ps = proj_ps.tile(
                            [128, 512], f32, tag="proj", name="qkh"
                        )
                        state["ps"] = ps
                        for k in range(4):
                            nc.tensor.matmul(
                                ps[:],
                                w_sb[:, m, k, :],
                                xt_sb[:, k, qsl],
                                start=(k == 0),
                                stop=False,
                            )

                    def h2():
                        ps = state.pop("ps")
                        for k in range(4, KT):
                            nc.tensor.matmul(
                                ps[:],
                                w_sb[:, m, k, :],
                                xt_sb[:, k, qsl],
                                start=False,
                                stop=(k == KT - 1),
                            )
                        for j in range(2):
                            h = 2 * m + j
                            nc.vector.tensor_scalar_add(
                                qk_sb[j * 64 : j * 64 + 64, qki, m, qsl],
                                ps[j * 64 : j * 64 + 64, :],
                                b_sb[:, h : h + 1],
                            )

                    return h1, h2

                # Warm the PE p-state during the initial DMA wait: the clock
                # ramps to full speed only after ~3us of continuous
                # execution, so burn that ramp on throwaway matmuls with no
                # input dependencies instead of on the first projections.
                junk = small.tile([128, 512], bf16, tag="junk")
                nc.vector.memset(junk[:], 0.0)
                for _ in range(10):
                    jp = proj_ps.tile([128, 512], f32, tag="proj", name="jp")
                    nc.tensor.matmul(
                        jp[:1, :], onescol[:], junk[:], start=True, stop=True
                    )

                # --- Unified software pipeline -------------------------------
                # Flat stream of 64 (block, qq) units, blocks B0..B7 =
                # (0,0)..(0,3),(1,0)..(1,3). At driver step g we emit the
                # scores+exp for stream position g while the attention
                # matmuls lag behind on their own schedule (exp tiles buffer
                # in SBUF). The lag starts at 8 units - shedding deferrable
                # PE work from the DMA/projection-heavy lead-in - and
                # catches up to 3 via double-attention steps in the middle
                # stretch where the exp stream is the binding engine anyway.
                # K/Q projection fillers sit at just-in-time exp-stream
                # steps; out-projections follow each at_sb completion.
                BLOCKS = [(0, 0), (0, 1), (0, 2), (0, 3)] + [
                    (1, qc) for qc in range(QC)
                ]
                pairs = {}

                def get_pair(bi):
                    if bi not in pairs:
                        pairs[bi] = AttnPair(*BLOCKS[bi])
                    return pairs[bi]

                def chunk_dma(c):
                    qsl = slice(c * 512, (c + 1) * 512)
                    if c == 0:
                        # Split the first x^T chunk and pull only the m=0
                        # halves of Wk/Wq so the first projection matmuls
                        # start as early as the DMA stream allows. Wv and
                        # the other consts are deferred off this critical
                        # chain; bk/bq slot between the big loads right
                        # before their bias adds need them.
                        nc.sync.dma_start(wk_sb[:, 0], wk_d[0])
                        nc.sync.dma_start(xt_sb[:, :4, qsl], x_d[c, :, :4, :])
                        nc.sync.dma_start(bk_sb[:], bk_d[:])
                        nc.sync.dma_start(wq_sb[:, 0], wq_d[0])
                        nc.sync.dma_start(xt_sb[:, 4:, qsl], x_d[c, :, 4:, :])
                        nc.sync.dma_start(bq_sb[:], bq_d[:])
                    else:
                        nc.sync.dma_start(xt_sb[:, :, qsl], x_d[c])
                    if c == 1:
                        nc.sync.dma_start(wk_sb[:, 1], wk_d[1])
                        nc.sync.dma_start(wv_sb[:], wv_d[:])
                    elif c == 2:
                        nc.sync.dma_start(wq_sb[:, 1], wq_d[1])

                def chunk_proj(c):
                    if c == 0:
                        # First chunk: interleave the K/Q matmuls per x-half
                        # so Q isn't serialized behind all of K, and the K
                        # bias adds overlap Q's second half on DVE.
                        psk = proj_ps.tile([128, 512], f32, tag="proj", name="psk")
                        psq = proj_ps.tile([128, 512], f32, tag="proj", name="psq")
                        for half in range(2):
                            for ps, w in ((psk, wk_sb), (psq, wq_sb)):
                                for k in range(4 * half, 4 * half + 4):
                                    nc.tensor.matmul(
                                        ps[:],
                                        w[:, 0, k, :],
                                        xt_sb[:, k, 0:512],
                                        start=(k == 0),
                                        stop=(k == KT - 1),
                                    )
                        for j in range(2):  # j-major: scores j0 needs only
                            for ps, b_sb, qki in (  # the two j0 adds
                                (psk, bk_sb, 1),
                                (psq, bq_sb, 0),
                            ):
                                nc.vector.tensor_scalar_add(
                                    qk_sb[j * 64 : j * 64 + 64, qki, 0, 0:512],
                                    ps[j * 64 : j * 64 + 64, :],
                                    b_sb[:, j : j + 1],
                                )
                        return
                    qk_proj(wk_sb, bk_sb, 1, 0, c)
                    if c == 1:
                        qk_proj(wq_sb, bq_sb, 0, 0, c)

                CH = {0: 0, 2: 1, 4: 2, 6: 3}
                pre_dma = {g: (lambda c=c: chunk_dma(c)) for g, c in CH.items()}

                def _late_consts():
                    nc.sync.dma_start(ident[:], id_d[:])
                    nc.sync.dma_start(wo_sb[:], wo_d[:])

                pre_dma[7] = _late_consts
                # Chunk K/Q m0 projections must precede the same step's
                # s_exp; V and the just-in-time K/Q m1 fillers have >= 1
                # step of deadline slack, so they are emitted AFTER the
                # s_exp - whenever PE is the laggard, scores jump the queue.
                pre_proj = {g: [lambda c=c: chunk_proj(c)] for g, c in CH.items()}
                post_proj = {}
                for s in range(ST):  # V singles, late but before their attn
                    post_proj.setdefault(s + 3, []).append(
                        lambda s=s: v_proj_st(s)
                    )
                # JIT K/Q m1 + deferred Q m0 projections, in halves. The
                # "pre" placements hit deadlines where the consumer's scores
                # sit in the same step's s_exp slot.
                for qki, m, qc, g1, g2, pre2 in [
                    (0, 0, 2, 13, 14, False),
                    (0, 0, 3, 20, 21, False),
                    (1, 1, 0, 27, 28, False),
                    (0, 1, 0, 29, 30, False),
                    (1, 1, 1, 31, 32, False),
                    (1, 1, 2, 34, 35, False),
                    (1, 1, 3, 36, 37, False),
                    (0, 1, 1, 38, 39, False),
                    (0, 1, 2, 45, 46, False),
                    (0, 1, 3, 52, 53, False),
                ]:
                    w_sb_, b_sb_ = (
                        (wq_sb, bq_sb) if qki == 0 else (wk_sb, bk_sb)
                    )
                    h1, h2 = KQ_halves(w_sb_, b_sb_, qki, m, qc)
                    post_proj.setdefault(g1, []).append(h1)
                    (pre_proj if pre2 else post_proj).setdefault(g2, []).append(
                        h2
                    )

                def OP(m, act_copy=False):
                    return lambda: out_proj_m(m, act_copy)

                fill = {}
                for i in range(12):  # B5..B7 odd-qq slots: out-proj 0..11
                    fill[40 + 2 * i + 1] = OP(i)
                followers = [OP(m, act_copy=True) for m in range(12, 16)]

                # Attention schedule: lag 12 initially (shedding deferrable
                # PE work out of the projection-heavy lead-in), catching up
                # to lag 3 via double-steps in the ACT-bound middle stretch.
                attn_sched = {}
                a = 0
                for g in range(14, 100):
                    if a >= 64:
                        break
                    doubled = g in (24, 25, 26, 33, 40, 41, 42, 43, 44, 49, 51)
                    for _ in range(2 if doubled else 1):
                        if a < 64:
                            attn_sched.setdefault(g, []).append(a)
                            a += 1
                last_step = max(attn_sched)

                for g in range(last_step + 1):
                    if g in pre_dma:
                        pre_dma[g]()
                    for au in attn_sched.get(g, []):
                        bi, qq = divmod(au, 8)
                        get_pair(bi).attn(qq)
                        f = fill.get(au)
                        if f:
                            f()
                        if qq == 7:
                            get_pair(bi).finish(
                                followers if bi == 7 else None,
                                act_assist=(bi == 7),
                            )
                    for h in pre_proj.get(g, []):
                        h()
                    if g < 64:
                        bi, qq = divmod(g, 8)
                        get_pair(bi).s_exp(qq)
                    for h in post_proj.get(g, []):
                        h()

    nc.compile()
    return nc


def _bf16(a):
    import concourse.mybir as mybir

    return np.ascontiguousarray(a, dtype=np.float32).astype(
        mybir.dt.np(mybir.dt.bfloat16)
    )


def shard_inputs(inputs):
    """Build the 8 per-core input maps: core c -> batch c//4, head-group c%4."""
    x = np.asarray(inputs["x"], dtype=np.float32)
    Wq = np.asarray(inputs["Wq"], dtype=np.float32)
    Wk = np.asarray(inputs["Wk"], dtype=np.float32)
    Wv = np.asarray(inputs["Wv"], dtype=np.float32)
    bq = np.asarray(inputs["bq"], dtype=np.float32)
    bk = np.asarray(inputs["bk"], dtype=np.float32)
    Wo = np.asarray(inputs["Wo"], dtype=np.float32)
    ident = np.eye(128, dtype=np.float32)

    def wslice(W, g):
        # [1024, 256] -> [MT, 128, KT, 128] (m-major, partition-major k-tiles)
        w = W[:, g * DQ : (g + 1) * DQ]
        return _bf16(w.reshape(KT, 128, MT, 128).transpose(2, 1, 0, 3))

    def wvslice(W, g):
        # [1024, 256] -> [128, KT, 256] (partition-major k-tiles)
        w = W[:, g * DQ : (g + 1) * DQ]
        return _bf16(w.reshape(KT, 128, DQ).transpose(1, 0, 2))

    def bcol(b, g):
        # [256] -> [64, 4]: per-head per-partition columns
        return np.ascontiguousarray(b[g * DQ : (g + 1) * DQ].reshape(HPC, DH).T)

    in_maps = []
    for c in range(NCORES):
        b, g = divmod(c, HPC)
        wo = Wo[g * DQ : (g + 1) * DQ, :]
        in_maps.append(
            {
                "x": _bf16(
                    x[b].T.reshape(KT, 128, QC, 512).transpose(2, 1, 0, 3)
                ),
                "wq": wslice(Wq, g),
                "wk": wslice(Wk, g),
                "wv": wvslice(Wv, g),
                "bq": bcol(bq, g),
                "bk": bcol(bk, g),
                "wo": _bf16(wo.reshape(MT, 128, DOUT).transpose(1, 0, 2)),
                "ident": _bf16(ident),
            }
        )
    return in_maps


_PROGRAM_CACHE = []


def run_on_hw(inputs, trace=False):
    from concourse.bass_utils import run_bass_kernel_spmd

    if not _PROGRAM_CACHE:
        _PROGRAM_CACHE.append(build_program(1))
    nc = _PROGRAM_CACHE[0]
    in_maps = shard_inputs(inputs)
    # trace=True needs the axon NTFF hook (antenv.axon_hooks), absent here.
    res = run_bass_kernel_spmd(nc, in_maps, list(range(NCORES)), trace=False)
    bo = np.asarray(inputs["bo"], dtype=np.float32)
    bv = np.asarray(inputs["bv"], dtype=np.float64)
    Wo = np.asarray(inputs["Wo"], dtype=np.float64)
    const = (bo.astype(np.float64) + bv @ Wo).astype(np.float32)
    out = np.zeros((B, S, DOUT), dtype=np.float32)
    for c in range(NCORES):
        out[c // HPC] += res.results[c]["out"]
    out += const
    return out, res


def kernel(**inputs):
    out, _ = run_on_hw(inputs, trace=False)
    return out
